# revision 8
# baseline (speedup 1.0000x reference)
"""Trainium2 Bass kernel for nn_AthenaSA: RMSNorm -> fused QKV -> RoPE ->
causal SDPA -> out_proj + residual, returning (out, present_k, present_v).

Sharding (8 cores): batch (2-way data parallel) x heads (4-way tensor
parallel).  Core c handles batch b=c//4 and heads [4g, 4g+4), g=c%4.  Each
core computes its 4 heads end-to-end; the out_proj partial sums are
reduce-scattered over each batch group of 4 cores, the residual is added to
the owned S/4 slice, and the host reassembles full outputs.

Compute is bf16 on the TensorEngine (fp32 PSUM accumulate); softmax sums and
normalization are fp32.  Softmax skips max-subtraction (scores are O(10) for
this problem's scale, safe in fp32 exp).

Attention layout trick: scores are computed TRANSPOSED (s_k on partitions,
s_q on free axis) so no transposes are needed anywhere: QK^T uses k_rot^T
tiles as the stationary operand, softmax denominators come from an extra
all-ones matmul accumulated alongside PV, and PV consumes v in natural
layout as the stationary operand, producing attn_out^T which feeds out_proj
directly.
"""
import math
import sys

import numpy as np

sys.path.insert(0, "/opt/trn_rl_repo")

import concourse.bass as bass  # noqa: E402
import concourse.tile as tile  # noqa: E402
from concourse import bacc, mybir  # noqa: E402
from concourse.bass_utils import run_bass_kernel_spmd  # noqa: E402

B, S, E, H, K, V = 2, 2048, 2048, 16, 128, 128
EPS = 1e-5
HL = 4            # heads per core
P = 128           # partitions
ET = E // P       # 16 e-tiles
ST = S // P       # 16 s-tiles
SB = 512          # s-block (psum free width)
NSB = S // SB     # 4 s-blocks
QC = HL * K       # 512 local qkv cols for each of q/k/v
F32 = mybir.dt.float32
BF16 = mybir.dt.bfloat16

_CACHE = {}
LAST_EXEC_NS = None
LAST_RESULTS = None


def _ensure_ntff_hook():
    """The image's antenv lacks axon_hooks; install an equivalent shim so
    run_bass_kernel_spmd(trace=True) can capture NTFF profiles."""
    import types
    try:
        from antenv.axon_hooks import get_axon_ntff_profile_hook  # noqa: F401
        return
    except ImportError:
        pass
    try:
        import antenv
        from trn_agent_boot.trn_boot import _ntff_profile_via_ctypes
        m = types.ModuleType("antenv.axon_hooks")
        m._hook = _ntff_profile_via_ctypes("/opt/axon/libaxon_pjrt.so")
        m.set_axon_ntff_profile_hook = lambda h: setattr(m, "_hook", h)
        m.get_axon_ntff_profile_hook = lambda: m._hook
        sys.modules["antenv.axon_hooks"] = m
        antenv.axon_hooks = m
    except Exception:
        pass


def build_graph(causal_tril: bool):
    nc = bacc.Bacc("TRN2", target_bir_lowering=False, debug=False, num_devices=8)

    embT = nc.dram_tensor("embT", [E, S], F32, kind="ExternalInput").ap()
    emb_res = nc.dram_tensor("emb_res", [NSB, P, E], F32, kind="ExternalInput").ap()
    w_qkv_t = nc.dram_tensor("w_qkv_t", [E, 3 * QC], F32, kind="ExternalInput").ap()
    w_out_s = nc.dram_tensor("w_out_s", [QC, E], F32, kind="ExternalInput").ap()
    w_norm_t = nc.dram_tensor("w_norm_t", [P, ET], F32, kind="ExternalInput").ap()
    cos_q = nc.dram_tensor("cos_q", [K, S], F32, kind="ExternalInput").ap()
    sin_q = nc.dram_tensor("sin_q", [K, S], F32, kind="ExternalInput").ap()
    cos_k = nc.dram_tensor("cos_k", [K, S], F32, kind="ExternalInput").ap()
    sin_k = nc.dram_tensor("sin_k", [K, S], F32, kind="ExternalInput").ap()
    if not causal_tril:
        # transposed mask maskT[s_k, s_q]
        maskT = nc.dram_tensor("maskT", [S, S], F32, kind="ExternalInput").ap()

    out_slice = nc.dram_tensor("out_slice", [NSB, P, E], F32, kind="ExternalOutput").ap()
    k_out = nc.dram_tensor("k_out", [HL, K, S], F32, kind="ExternalOutput").ap()
    v_out = nc.dram_tensor("v_out", [S, QC], F32, kind="ExternalOutput").ap()

    inv_sqrt_k = 1.0 / math.sqrt(K)

    with tile.TileContext(nc) as tc:
        with (
            tc.tile_pool(name="dram", bufs=1, space="DRAM") as dram,
            tc.tile_pool(name="persist", bufs=1) as persist,
            tc.tile_pool(name="qkvout", bufs=1) as qkvout,
        ):
            partial = dram.tile([NSB, SB, E], F32, tag="partial")
            rs_out_d = dram.tile([NSB, P, E], F32, tag="rs_out")

            ones_bf = persist.tile([P, P], BF16, tag="ones")
            nc.vector.memset(ones_bf, 1.0)
            eps_t = persist.tile([P, 1], F32, tag="eps")
            nc.vector.memset(eps_t, EPS)
            wn_sb = persist.tile([P, ET], F32, tag="wn")
            nc.sync.dma_start(out=wn_sb, in_=w_norm_t[:, :])

            # q/k arrive transposed [head_dim, S]; rotated IN PLACE later.
            q_bf = [qkvout.tile([P, S], BF16, tag=f"qbf{j}", name=f"qbf{j}") for j in range(HL)]
            k_bf = [qkvout.tile([P, S], BF16, tag=f"kbf{j}", name=f"kbf{j}") for j in range(HL)]
            # v natural [s, 4*128]
            v_bf = [qkvout.tile([P, QC], BF16, tag=f"vbf{m}", name=f"vbf{m}") for m in range(ST)]

            # ============ Phases A+B: norm + QKV (x resident) ============
            with tc.tile_pool(name="xpool", bufs=1) as xpool:
                x = []
                for e in range(ET):
                    xt = xpool.tile([P, S], BF16, tag=f"x{e}")
                    nc.gpsimd.dma_start(out=xt, in_=embT[e * P:(e + 1) * P, :])
                    x.append(xt)

                # ---- stats + normalization
                with tc.tile_pool(name="sqpool", bufs=3) as sqpool, \
                     tc.tile_pool(name="scalep", bufs=1) as scalep, \
                     tc.tile_pool(name="mspool", bufs=1, space="PSUM") as mspool:
                    ms_ps = [mspool.tile([P, SB], F32, tag=f"ms{q}", name=f"ms{q}")
                             for q in range(NSB)]
                    for e in range(ET):
                        sq = sqpool.tile([P, S], BF16, tag="sq")
                        nc.vector.tensor_mul(sq, x[e], x[e])
                        for q in range(NSB):
                            nc.tensor.matmul(ms_ps[q][:, :], ones_bf,
                                             sq[:, q * SB:(q + 1) * SB],
                                             start=(e == 0), stop=(e == ET - 1))

                    scale_bc = scalep.tile([P, S], BF16, tag="scale")
                    for q in range(NSB):
                        rms = scalep.tile([P, SB], F32, tag=f"rms{q}",
                                          name=f"rms{q}")
                        nc.scalar.activation(out=rms,
                                             in_=ms_ps[q][:, :],
                                             func=mybir.ActivationFunctionType.Sqrt,
                                             bias=eps_t, scale=1.0 / E)
                        with nc.allow_low_precision(
                                reason="rms scale is O(1); bf16 is plenty"):
                            nc.vector.reciprocal(
                                out=scale_bc[:, q * SB:(q + 1) * SB], in_=rms)
                    for e in range(ET):
                        nc.vector.tensor_mul(x[e], x[e], scale_bc)
                        nc.vector.tensor_scalar_mul(x[e], x[e], wn_sb[:, e:e + 1])

                # ---- QKV projections; w_qkv streamed in 512-col slices
                with tc.tile_pool(name="qkps", bufs=3, space="PSUM") as qkps, \
                     tc.tile_pool(name="qkcp", bufs=3) as qkcp:
                    for sl in range(3):          # 0: q cols, 1: k cols, 2: v cols
                        with tc.tile_pool(name=f"wp{sl}", bufs=1) as wpool:
                            w_bf = []
                            for e in range(ET):
                                wt = wpool.tile([P, QC], BF16, tag=f"w{e}")
                                nc.gpsimd.dma_start(
                                    out=wt,
                                    in_=w_qkv_t[e * P:(e + 1) * P,
                                                sl * QC:(sl + 1) * QC])
                                w_bf.append(wt)
                            if sl < 2:
                                # weight-stationary -> [feat, s] transposed
                                dsts = q_bf if sl == 0 else k_bf
                                for j in range(HL):
                                    for q in range(NSB):
                                        ps = qkps.tile([P, SB], F32, tag="qk")
                                        for e in range(ET):
                                            nc.tensor.matmul(
                                                ps[:, :],
                                                w_bf[e][:, j * P:(j + 1) * P],
                                                x[e][:, q * SB:(q + 1) * SB],
                                                start=(e == 0), stop=(e == ET - 1))
                                        nc.vector.tensor_copy(
                                            out=dsts[j][:, q * SB:(q + 1) * SB],
                                            in_=ps[:, :])
                                if sl == 1:
                                    for j in range(HL):
                                        # cast bf16 -> f32 during DMA
                                        nc.gpsimd.dma_start(out=k_out[j],
                                                            in_=k_bf[j])
                            else:
                                # v: normed-stationary -> natural [s, 512]
                                for m in range(ST):
                                    ps = qkps.tile([P, QC], F32, tag="qk")
                                    for e in range(ET):
                                        nc.tensor.matmul(
                                            ps[:, :],
                                            x[e][:, m * P:(m + 1) * P],
                                            w_bf[e][:, :],
                                            start=(e == 0), stop=(e == ET - 1))
                                    vf = qkcp.tile([P, QC], F32, tag="vf")
                                    nc.vector.tensor_copy(out=vf, in_=ps[:, :])
                                    nc.sync.dma_start(
                                        out=v_out[m * P:(m + 1) * P, :], in_=vf)
                                    nc.vector.tensor_copy(out=v_bf[m], in_=vf)

            # ============ Phase C: RoPE (in place on q_bf/k_bf) ============
            half = K // 2
            with tc.tile_pool(name="trig", bufs=1) as trig, \
                 tc.tile_pool(name="ropetmp", bufs=2) as ropetmp:
                cq = trig.tile([K, S], BF16, tag="cq")
                sq_ = trig.tile([K, S], BF16, tag="sq_")
                ck = trig.tile([K, S], BF16, tag="ck")
                sk = trig.tile([K, S], BF16, tag="sk")
                nc.gpsimd.dma_start(out=cq, in_=cos_q[:, :])
                nc.gpsimd.dma_start(out=sq_, in_=sin_q[:, :])
                nc.gpsimd.dma_start(out=ck, in_=cos_k[:, :])
                nc.gpsimd.dma_start(out=sk, in_=sin_k[:, :])

                for j in range(HL):
                    for src, c_, s_ in ((q_bf[j], cq, sq_), (k_bf[j], ck, sk)):
                        t1 = ropetmp.tile([K, S], BF16, tag="t1")
                        nc.vector.tensor_copy(out=t1[0:half, :], in_=src[half:K, :])
                        nc.vector.tensor_copy(out=t1[half:K, :], in_=src[0:half, :])
                        nc.vector.tensor_mul(t1, t1, s_)
                        nc.vector.tensor_mul(src, src, c_)   # in place
                        nc.vector.tensor_add(src, src, t1)   # in place

            # ============ Phase D: attention + out_proj + RS + residual ====
            with tc.tile_pool(name="wout", bufs=1) as woutp, \
                 tc.tile_pool(name="maskp", bufs=1) as maskp, \
                 tc.tile_pool(name="attn", bufs=1) as attnp, \
                 tc.tile_pool(name="expp", bufs=4) as expp, \
                 tc.tile_pool(name="smallp", bufs=4) as smallp, \
                 tc.tile_pool(name="qk2ps", bufs=2, space="PSUM") as qk2ps, \
                 tc.tile_pool(name="pvps", bufs=2, space="PSUM") as pvps, \
                 tc.tile_pool(name="sumps", bufs=2, space="PSUM") as sumps, \
                 tc.tile_pool(name="opps", bufs=2, space="PSUM") as opps, \
                 tc.tile_pool(name="finp", bufs=2) as finp:

                w_out_bf = []
                for j in range(HL):
                    wt = woutp.tile([P, E], BF16, tag=f"wo{j}")
                    nc.gpsimd.dma_start(out=wt, in_=w_out_s[j * P:(j + 1) * P, :])
                    w_out_bf.append(wt)

                if causal_tril:
                    masks = []
                    ones_m = maskp.tile([P, SB], BF16, tag="ones_m")
                    nc.vector.memset(ones_m, 1.0)
                    for d in range(NSB):
                        mk = maskp.tile([P, SB], BF16, tag=f"mk{d}")
                        nc.gpsimd.affine_select(
                            out=mk, in_=ones_m, pattern=[[1, SB]],
                            compare_op=mybir.AluOpType.is_ge, fill=0.0,
                            base=-P * d, channel_multiplier=-1)
                        masks.append(mk)
                else:
                    mask_bf = []
                    for t in range(ST):
                        mt = maskp.tile([P, S], BF16, tag=f"mask{t}")
                        nc.gpsimd.dma_start(out=mt, in_=maskT[t * P:(t + 1) * P, :])
                        mask_bf.append(mt)

                attn_bf = [[attnp.tile([P, SB], BF16, tag=f"attn{j}_{q}", name=f"attn{j}_{q}")
                            for q in range(NSB)] for j in range(HL)]

                for Q in range(NSB):
                    nt = 4 * Q + 4 if causal_tril else ST
                    for j in range(HL):
                        pv = pvps.tile([P, SB], F32, tag="pv")
                        sm = sumps.tile([P, SB], F32, tag="sm")
                        for t in range(nt):
                            qk = qk2ps.tile([P, SB], F32, tag="qk2")
                            nc.tensor.matmul(
                                qk[:, :],
                                k_bf[j][:, t * P:(t + 1) * P],
                                q_bf[j][:, Q * SB:(Q + 1) * SB],
                                start=True, stop=True)
                            ex = expp.tile([P, SB], BF16, tag="ex")
                            nc.scalar.activation(
                                out=ex, in_=qk[:, :],
                                func=mybir.ActivationFunctionType.Exp,
                                scale=inv_sqrt_k)
                            if causal_tril:
                                if t >= 4 * Q:
                                    nc.vector.tensor_mul(ex, ex, masks[t - 4 * Q])
                            else:
                                nc.vector.tensor_mul(
                                    ex, ex, mask_bf[t][:, Q * SB:(Q + 1) * SB])
                            nc.tensor.matmul(pv[:, :],
                                             v_bf[t][:, j * K:(j + 1) * K], ex,
                                             start=(t == 0), stop=(t == nt - 1))
                            nc.tensor.matmul(sm[:, :], ones_bf, ex,
                                             start=(t == 0), stop=(t == nt - 1))
                        rc = smallp.tile([P, SB], F32, tag="rc")
                        nc.vector.reciprocal(out=rc, in_=sm[:, :])
                        nc.vector.tensor_mul(attn_bf[j][Q], pv[:, :], rc)

                    # out_proj rows [512Q, 512Q+512)
                    for m in range(NSB):
                        for eb in range(NSB):
                            op = opps.tile([P, SB], F32, tag="op")
                            for j in range(HL):
                                nc.tensor.matmul(
                                    op[:, :],
                                    attn_bf[j][Q][:, m * P:(m + 1) * P],
                                    w_out_bf[j][:, eb * SB:(eb + 1) * SB],
                                    start=(j == 0), stop=(j == HL - 1))
                            ob = smallp.tile([P, SB], F32, tag="ob")
                            nc.vector.tensor_copy(out=ob, in_=op[:, :])
                            nc.sync.dma_start(
                                out=partial[Q, m * P:(m + 1) * P,
                                            eb * SB:(eb + 1) * SB],
                                in_=ob)
                    nc.gpsimd.collective_compute(
                        "ReduceScatter",
                        mybir.AluOpType.add,
                        ins=[partial[Q]],
                        outs=[rs_out_d[Q]],
                        replica_groups=[[0, 1, 2, 3], [4, 5, 6, 7]],
                    )
                    fin = finp.tile([P, E], F32, tag="fin")
                    nc.sync.dma_start(out=fin, in_=rs_out_d[Q])
                    res = finp.tile([P, E], F32, tag="res")
                    nc.sync.dma_start(out=res, in_=emb_res[Q])
                    nc.vector.tensor_add(fin, fin, res)
                    nc.sync.dma_start(out=out_slice[Q], in_=fin)

    nc.finalize()
    return nc


def _prep_inputs(embeddings, cos_buffer, sin_buffer, causal_buffer,
                 w_norm, w_qkv, w_out, causal_tril):
    ks = K * H
    cq = np.ascontiguousarray(np.asarray(cos_buffer)[0, 0, 0].T)
    sq = np.ascontiguousarray(np.asarray(sin_buffer)[0, 0, 0].T)
    ck = np.ascontiguousarray(np.asarray(cos_buffer)[1, 0, 0].T)
    sk = np.ascontiguousarray(np.asarray(sin_buffer)[1, 0, 0].T)
    wn_t = np.ascontiguousarray(np.asarray(w_norm).reshape(ET, P).T)
    if not causal_tril:
        maskT = np.ascontiguousarray(
            np.asarray(causal_buffer)[0, 0].T.astype(np.float32))

    in_maps = []
    for c in range(8):
        b, g = c // 4, c % 4
        emb = np.asarray(embeddings)[b]
        embT = np.ascontiguousarray(emb.T)
        emb_res = np.stack([emb[SB * Q + P * g: SB * Q + P * (g + 1), :]
                            for Q in range(NSB)])
        wq = np.asarray(w_qkv)
        w_qkv_t = np.ascontiguousarray(np.concatenate([
            wq[:, QC * g: QC * (g + 1)],
            wq[:, ks + QC * g: ks + QC * (g + 1)],
            wq[:, 2 * ks + QC * g: 2 * ks + QC * (g + 1)],
        ], axis=1))
        w_out_sl = np.ascontiguousarray(np.asarray(w_out)[QC * g: QC * (g + 1), :])
        m = dict(embT=embT, emb_res=np.ascontiguousarray(emb_res),
                 w_qkv_t=w_qkv_t, w_out_s=w_out_sl, w_norm_t=wn_t,
                 cos_q=cq, sin_q=sq, cos_k=ck, sin_k=sk)
        if not causal_tril:
            m["maskT"] = maskT
        in_maps.append(m)
    return in_maps


def kernel(embeddings, cos_buffer, sin_buffer, causal_buffer,
           w_norm, w_qkv, w_out, trace=False):
    global LAST_EXEC_NS, LAST_RESULTS
    causal = np.asarray(causal_buffer)[0, 0]
    causal_tril = bool(np.array_equal(causal, np.tril(np.ones((S, S), bool))))

    if causal_tril not in _CACHE:
        _CACHE[causal_tril] = build_graph(causal_tril)
    nc = _CACHE[causal_tril]

    in_maps = _prep_inputs(embeddings, cos_buffer, sin_buffer, causal_buffer,
                           w_norm, w_qkv, w_out, causal_tril)
    if trace:
        _ensure_ntff_hook()
    res = run_bass_kernel_spmd(nc, in_maps, core_ids=list(range(8)), trace=trace)
    LAST_EXEC_NS = res.exec_time_ns
    LAST_RESULTS = res

    out = np.empty((B, S, E), np.float32)
    present_k = np.empty((B, H, S, K), np.float32)
    present_v = np.empty((B, H, S, V), np.float32)
    for c in range(8):
        b, g = c // 4, c % 4
        r = res.results[c]
        osl = r["out_slice"]
        for Q in range(NSB):
            out[b, SB * Q + P * g: SB * Q + P * (g + 1), :] = osl[Q]
        ko = r["k_out"]          # [HL, K, S]
        vo = r["v_out"]          # [S, QC]
        for j in range(HL):
            present_k[b, HL * g + j] = ko[j].T
            present_v[b, HL * g + j] = vo[:, K * j: K * (j + 1)]
    return out, present_k, present_v
